# revision 9
# baseline (speedup 1.0000x reference)
"""Fused LayerNorm + multi-head attention Trainium2 kernel, 8-core SPMD.

Problem: x[4, 2048, 768] -> LN -> QKV (w_qkv[2304, 768]) -> 12-head attention
         -> out proj (w_out[768, 768] + b_out). f32 I/O, bf16 tensor-engine compute.

Sharding: core c handles batch b=c//2, query-half g=c%2 (1024 queries each).
Each core receives the FULL (rotated) sequence of its batch so K/V are computed
locally -- no collectives. The token order is rotated per-core so the core's own
query chunk is always columns [0, 1024) => identical SPMD program on all cores.

On-core layout: channel-major. LayerNorm mean/bias are folded into the QKV
matmul via two appended contraction rows; the softmax denominator is computed by
an appended ones-column in the AV matmul's stationary operand.
"""

import numpy as np
import ml_dtypes

import concourse.bass as bass
import concourse.tile as tile
from concourse import bacc, mybir
from concourse.bass_utils import run_bass_kernel_spmd

F32 = mybir.dt.float32
BF16 = mybir.dt.bfloat16
AF = mybir.ActivationFunctionType
ALU = mybir.AluOpType

DIM = 768
HEADS = 12
B, N = 4, 2048
D = 64          # head dim
NQ = 1024       # queries per core
CT = 6          # 768 / 128 channel tiles
NT = 16         # 2048 / 128 token tiles
HP = 6          # head pairs

LAST = None  # BassKernelResults of the most recent run (for test harness)
_NC = None


def build(debug=False):
    nc = bacc.Bacc("TRN2", target_bir_lowering=False, debug=False, num_devices=8)

    xT = nc.dram_tensor("xT", [DIM, N], F32, kind="ExternalInput")
    wqkvT = nc.dram_tensor("wqkvT", [DIM + 2, 3 * DIM], BF16, kind="ExternalInput")
    woutT = nc.dram_tensor("woutT", [DIM + 1, DIM], BF16, kind="ExternalInput")
    outT = nc.dram_tensor("outT", [DIM, NQ], F32, kind="ExternalOutput")
    if debug:
        d_rows = nc.dram_tensor("d_rows", [2, N], F32, kind="ExternalOutput")
        d_xtil = nc.dram_tensor("d_xtil", [128, CT, N], BF16, kind="ExternalOutput")
        d_KT = nc.dram_tensor("d_KT", [128, CT, N], BF16, kind="ExternalOutput")
        d_QT = nc.dram_tensor("d_QT", [128, CT, NQ], BF16, kind="ExternalOutput")
        d_V4 = nc.dram_tensor("d_V4", [128, NT, HEADS, D + 1], BF16, kind="ExternalOutput")
        d_AO = nc.dram_tensor("d_AO", [128, CT, NQ], BF16, kind="ExternalOutput")

    with tile.TileContext(nc) as tc:
        with (
            tc.tile_pool(name="persist", bufs=1) as P1,
            tc.tile_pool(name="work", bufs=2) as PW,
            tc.tile_pool(name="rows", bufs=2) as PR,
            tc.tile_pool(name="psA", bufs=1, space="PSUM") as PSA,
            tc.tile_pool(name="psB", bufs=2, space="PSUM") as PSB,
        ):
            # ---- persistent SBUF tensors ----
            wq = P1.tile([128, CT, 3 * DIM], BF16)       # W'' rows 0..767
            wex = P1.tile([2, 3 * DIM], BF16)            # W'' rows 768..769
            WO = P1.tile([128, CT, DIM], BF16)           # w_out^T  (f-major tiles)
            wob = P1.tile([1, DIM], BF16)                # b_out row
            xtil = P1.tile([128, CT, N], BF16, tag="big_a")   # x~ = x^T * rstd
            xex = P1.tile([2, N], BF16)                  # x~ rows 768 (-mu*rstd), 769 (1)
            KT = P1.tile([128, CT, N], BF16)             # K^T channel-major
            QT = P1.tile([128, CT, NQ], BF16)            # Q^T channel-major
            V4 = P1.tile([128, NT, HEADS, D + 1], BF16)  # V token-major + ones col
            ones1 = P1.tile([128, 1], F32)               # f32 ones column (stats lhsT)
            ones1b = P1.tile([128, 1], BF16)             # bf16 ones column
            onesr = P1.tile([1, 128], F32)               # f32 ones row (bcast lhsT)
            onesI = P1.tile([1, NQ], BF16)               # bf16 ones row (bias rhs)
            epsc = P1.tile([1, 1], F32)

            nc.vector.memset(epsc[:], 1e-5)
            nc.vector.memset(ones1[:], 1.0)
            nc.vector.memset(ones1b[:], 1.0)
            nc.vector.memset(onesr[:], 1.0)
            nc.vector.memset(onesI[:], 1.0)
            # row 1 must stay 1.0; row 0 is overwritten with -mu*rstd in phase B
            nc.vector.memset(xex[:, :], 1.0)
            nc.vector.memset(V4[:, :, :, D : D + 1], 1.0)

            # ---- weight DMAs ----
            for ct in range(CT):
                nc.sync.dma_start(wq[:, ct, :], wqkvT[ct * 128 : (ct + 1) * 128, :])
                nc.sync.dma_start(WO[:, ct, :], woutT[ct * 128 : (ct + 1) * 128, :])
            nc.sync.dma_start(wex[:], wqkvT[DIM : DIM + 2, :])
            nc.sync.dma_start(wob[:], woutT[DIM : DIM + 1, :])

            # ---- phase A: token stats (sum, sumsq) via ones-matmul reduction ----
            ps_stats = PSA.tile([128, N], F32, tag="psA")
            xins = []
            for ct in range(CT):
                xin = PW.tile([128, N], F32, tag="xin", name=f"xinA_{ct}")
                nc.sync.dma_start(xin[:], xT[ct * 128 : (ct + 1) * 128, :])
                xsq = PW.tile([128, N], BF16, tag="xsq")
                nc.scalar.activation(xsq[:], xin[:], AF.Square)
                for jc in range(4):
                    sl = slice(jc * 512, (jc + 1) * 512)
                    nc.tensor.matmul(
                        ps_stats[0:1, sl], ones1[:], xin[:, sl],
                        start=(ct == 0), stop=(ct == CT - 1),
                    )
                    nc.tensor.matmul(
                        ps_stats[64:65, sl], ones1b[:], xsq[:, sl],
                        start=(ct == 0), stop=(ct == CT - 1),
                    )

            r_a = PR.tile([1, N], F32, tag="row")   # E[x^2] -> var -> rstd
            r_b = PR.tile([1, N], F32, tag="row")   # mu -> mu*rstd
            nc.vector.tensor_scalar_mul(r_a[:], ps_stats[64:65, :], 1.0 / DIM)
            nc.vector.tensor_scalar_mul(r_b[:], ps_stats[0:1, :], 1.0 / DIM)
            # mu^2 staged through PSUM so only one DVE op reads PSUM at a time
            nc.vector.tensor_tensor(ps_stats[32:33, :], r_b[:], r_b[:], ALU.mult)
            nc.vector.tensor_tensor(r_a[:], r_a[:], ps_stats[32:33, :], ALU.subtract)
            # rstd = exp(-0.5 * ln(var + eps))  (ln+exp share one ACT table set)
            nc.scalar.activation(r_a[:], r_a[:], AF.Ln, bias=epsc[:])
            nc.scalar.activation(r_a[:], r_a[:], AF.Exp, scale=-0.5)
            if debug:
                nc.sync.dma_start(d_rows[0:1, :], r_a[:])
                nc.sync.dma_start(d_rows[1:2, :], r_b[:])

            # ---- phase B: x~ = x * rstd (bf16), plus folded-LN extra rows ----
            ps_rb = PSA.tile([128, N], F32, tag="psA")
            for jc in range(4):
                sl = slice(jc * 512, (jc + 1) * 512)
                nc.tensor.matmul(ps_rb[:, sl], onesr[:], r_a[:, sl])
            for ct in range(CT):
                xin = PW.tile([128, N], F32, tag="xin", name=f"xinB_{ct}")
                nc.sync.dma_start(xin[:], xT[ct * 128 : (ct + 1) * 128, :])
                nc.vector.tensor_tensor(xtil[:, ct, :], xin[:], ps_rb[:], ALU.mult)
            nc.vector.tensor_tensor(r_b[:], r_b[:], r_a[:], ALU.mult)
            nc.vector.tensor_scalar_mul(xex[0:1, :], r_b[:], -1.0)
            if debug:
                for ct in range(CT):
                    nc.sync.dma_start(d_xtil[:, ct, :], xtil[:, ct, :])

            # ---- phase C: QKV projections ----
            # K then Q (channel-major): out[f, n] ; lhsT = W'' tile, rhs = x~
            for ft in range(CT):
                fsl = slice(DIM + ft * 128, DIM + (ft + 1) * 128)
                for nh in range(2):
                    acc = PSB.tile([128, 1024], F32, tag="psB")
                    for ct in range(CT + 1):
                        for s in range(2):
                            nsl = slice(nh * 1024 + s * 512, nh * 1024 + (s + 1) * 512)
                            nc.tensor.matmul(
                                acc[:, s * 512 : (s + 1) * 512],
                                wq[:, ct, fsl] if ct < CT else wex[:, fsl],
                                xtil[:, ct, nsl] if ct < CT else xex[:, nsl],
                                start=(ct == 0), stop=(ct == CT),
                            )
                    nc.vector.tensor_copy(KT[:, ft, nh * 1024 : (nh + 1) * 1024], acc[:])
            for ft in range(CT):
                fsl = slice(ft * 128, (ft + 1) * 128)
                acc = PSB.tile([128, 1024], F32, tag="psB")
                for ct in range(CT + 1):
                    for s in range(2):
                        nsl = slice(s * 512, (s + 1) * 512)
                        nc.tensor.matmul(
                            acc[:, nsl],
                            wq[:, ct, fsl] if ct < CT else wex[:, fsl],
                            xtil[:, ct, nsl] if ct < CT else xex[:, nsl],
                            start=(ct == 0), stop=(ct == CT),
                        )
                nc.vector.tensor_copy(QT[:, ft, :], acc[:])
            # V (token-major): out[n, f] ; lhsT = x~ tile, rhs = W'' v-columns
            for nt in range(NT):
                nsl = slice(nt * 128, (nt + 1) * 128)
                acc = PSB.tile([128, 1024], F32, tag="psB")
                for ct in range(CT + 1):
                    # matmul output must not straddle a 2KB PSUM bank: 768 = 512 + 256
                    for lo, sz in ((0, 512), (512, 256)):
                        fsl = slice(2 * DIM + lo, 2 * DIM + lo + sz)
                        nc.tensor.matmul(
                            acc[:, lo : lo + sz],
                            xtil[:, ct, nsl] if ct < CT else xex[:, nsl],
                            wq[:, ct, fsl] if ct < CT else wex[:, fsl],
                            start=(ct == 0), stop=(ct == CT),
                        )
                nc.vector.tensor_copy(
                    V4[:, nt, :, 0:D],
                    acc[:, 0:DIM].rearrange("p (h d) -> p h d", h=HEADS),
                )

            if debug:
                for ct in range(CT):
                    nc.sync.dma_start(d_KT[:, ct, :], KT[:, ct, :])
                    nc.sync.dma_start(d_QT[:, ct, :], QT[:, ct, :])
                for nt in range(NT):
                    nc.sync.dma_start(d_V4[:, nt, :, :], V4[:, nt, :, :])

            # attention output, same f-major tile layout the out-proj consumes
            AO = P1.tile([128, CT, NQ], BF16, tag="big_a", name="AO")

            # ---- phase D: attention, one head-pair at a time ----
            for hp in range(HP):
                U0 = PSB.tile([128, 1024], F32, tag="psB", name=f"U0_{hp}")
                U1 = PSB.tile([128, 1024], F32, tag="psB", name=f"U1_{hp}")
                for jt in range(NT):
                    jsl = slice(jt * 128, (jt + 1) * 128)
                    sp = PSA.tile([128, N], F32, tag="psA", name=f"sp_{hp}_{jt}")
                    for s in range(2):
                        ssl = slice(s * 512, (s + 1) * 512)
                        nc.tensor.matmul(
                            sp[:, ssl], KT[0:64, hp, jsl], QT[0:64, hp, ssl],
                            start=True, stop=True,
                        )
                        nc.tensor.matmul(
                            sp[:, 1024 + s * 512 : 1024 + (s + 1) * 512],
                            KT[64:128, hp, jsl], QT[64:128, hp, ssl],
                            start=True, stop=True,
                        )
                    ET = PW.tile([128, N], BF16, tag="ET")
                    nc.scalar.activation(ET[:], sp[:], AF.Exp, scale=float(D) ** -0.5)
                    for s in range(2):
                        ssl = slice(s * 512, (s + 1) * 512)
                        nc.tensor.matmul(
                            U0[0 : D + 1, ssl], V4[:, jt, 2 * hp, :], ET[:, ssl],
                            start=(jt == 0), stop=(jt == NT - 1),
                        )
                        nc.tensor.matmul(
                            U1[0 : D + 1, ssl], V4[:, jt, 2 * hp + 1, :],
                            ET[:, 1024 + s * 512 : 1024 + (s + 1) * 512],
                            start=(jt == 0), stop=(jt == NT - 1),
                        )
                # normalize: r = 1/denominator, broadcast to 64 partitions via matmul
                rr = PR.tile([1, N], F32, tag="row", name=f"rr_{hp}")
                nc.vector.reciprocal(rr[0:1, 0:1024], U0[D : D + 1, :])
                nc.vector.reciprocal(rr[0:1, 1024:2048], U1[D : D + 1, :])
                rbp = PSA.tile([128, N], F32, tag="psA", name=f"rbp_{hp}")
                for s in range(4):
                    nc.tensor.matmul(
                        rbp[0:64, s * 512 : (s + 1) * 512],
                        onesr[:, 0:64],
                        rr[0:1, s * 512 : (s + 1) * 512],
                        start=True, stop=True,
                    )
                rbB = PW.tile([64, N], BF16, tag="rbB")
                nc.vector.tensor_copy(rbB[:], rbp[0:64, :])
                nc.vector.tensor_tensor(
                    AO[0:64, hp, :], U0[0:D, :], rbB[:, 0:1024], ALU.mult
                )
                AOtmp = PW.tile([64, NQ], BF16, tag="AOtmp", name=f"AOtmp_{hp}")
                nc.vector.tensor_tensor(
                    AOtmp[:], U1[0:D, :], rbB[:, 1024:2048], ALU.mult
                )
                nc.sync.dma_start(AO[64:128, hp, :], AOtmp[:])

            if debug:
                for ct in range(CT):
                    nc.sync.dma_start(d_AO[:, ct, :], AO[:, ct, :])

            # ---- phase E: output projection + bias ----
            for ot in range(CT):
                osl = slice(ot * 128, (ot + 1) * 128)
                po = PSB.tile([128, 1024], F32, tag="psB", name=f"po_{ot}")
                for s in range(2):
                    ssl = slice(s * 512, (s + 1) * 512)
                    nc.tensor.matmul(
                        po[:, ssl], wob[:, osl], onesI[:, ssl],
                        start=True, stop=False,
                    )
                    for ft in range(CT):
                        nc.tensor.matmul(
                            po[:, ssl], WO[:, ft, osl], AO[:, ft, ssl],
                            start=False, stop=(ft == CT - 1),
                        )
                outsb = PW.tile([128, 1024], F32, tag="outsb")
                nc.vector.tensor_copy(outsb[:], po[:])
                nc.sync.dma_start(outT[osl, :], outsb[:])

    nc.finalize()
    return nc


def _get_nc():
    global _NC
    if _NC is None:
        import os
        _NC = build(debug=os.environ.get("KDEBUG", "0") == "1")
    return _NC


def kernel(x, ln_w, ln_b, w_qkv, w_out, b_out):
    global LAST
    x = np.asarray(x, dtype=np.float32)
    ln_w = np.asarray(ln_w, dtype=np.float32)
    ln_b = np.asarray(ln_b, dtype=np.float32)
    w_qkv = np.asarray(w_qkv, dtype=np.float32)
    w_out = np.asarray(w_out, dtype=np.float32)
    b_out = np.asarray(b_out, dtype=np.float32)

    bf16 = ml_dtypes.bfloat16
    # W'' = [ (w_qkv * ln_w)^T ; rowsum of (w_qkv*ln_w) ; w_qkv @ ln_b ]
    wprime = w_qkv * ln_w[None, :]
    wqkvT = np.concatenate(
        [wprime.T, wprime.sum(axis=1)[None, :], (w_qkv @ ln_b)[None, :]], axis=0
    ).astype(bf16)
    woutT = np.concatenate([w_out.T, b_out[None, :]], axis=0).astype(bf16)

    in_maps = []
    for c in range(8):
        b, g = c // 2, c % 2
        order = np.r_[g * NQ : (g + 1) * NQ, (1 - g) * NQ : (2 - g) * NQ]
        xTc = np.ascontiguousarray(x[b][order].T)
        in_maps.append({"xT": xTc, "wqkvT": wqkvT, "woutT": woutT})

    nc = _get_nc()
    LAST = run_bass_kernel_spmd(nc, in_maps, core_ids=list(range(8)))

    out = np.empty((B, N, DIM), dtype=np.float32)
    for c in range(8):
        b, g = c // 2, c % 2
        out[b, g * NQ : (g + 1) * NQ, :] = LAST.results[c]["outT"].T
    return out


# revision 11
# speedup vs baseline: 1.1940x; 1.1940x over previous
"""Fused LayerNorm + multi-head attention Trainium2 kernel, 8-core SPMD.

Problem: x[4, 2048, 768] -> LN -> QKV (w_qkv[2304, 768]) -> 12-head attention
         -> out proj (w_out[768, 768] + b_out). f32 I/O, bf16 tensor-engine compute.

Sharding: core c handles batch b=c//2, query-half g=c%2 (1024 queries each).
Each core receives the FULL (rotated) sequence of its batch so K/V are computed
locally -- no collectives. The token order is rotated per-core so the core's own
query chunk is always columns [0, 1024) => identical SPMD program on all cores.

On-core layout: channel-major. LayerNorm mean/bias are folded into the QKV
matmul via two appended contraction rows; the softmax denominator is computed by
an appended ones-column in the AV matmul's stationary operand.
"""

import numpy as np
import ml_dtypes

import concourse.bass as bass
import concourse.tile as tile
from concourse import bacc, mybir
from concourse.bass_utils import run_bass_kernel_spmd

F32 = mybir.dt.float32
BF16 = mybir.dt.bfloat16
AF = mybir.ActivationFunctionType
ALU = mybir.AluOpType

DIM = 768
HEADS = 12
B, N = 4, 2048
D = 64          # head dim
NQ = 1024       # queries per core
CT = 6          # 768 / 128 channel tiles
NT = 16         # 2048 / 128 token tiles
HP = 6          # head pairs

LAST = None  # BassKernelResults of the most recent run (for test harness)
_NC = None


def build(debug=False):
    nc = bacc.Bacc("TRN2", target_bir_lowering=False, debug=False, num_devices=8)

    xT = nc.dram_tensor("xT", [DIM, N], F32, kind="ExternalInput")
    wqkvT = nc.dram_tensor("wqkvT", [DIM + 2, 3 * DIM], BF16, kind="ExternalInput")
    woutT = nc.dram_tensor("woutT", [DIM + 1, DIM], BF16, kind="ExternalInput")
    outT = nc.dram_tensor("outT", [DIM, NQ], F32, kind="ExternalOutput")
    if debug:
        d_rows = nc.dram_tensor("d_rows", [2, N], F32, kind="ExternalOutput")
        d_xtil = nc.dram_tensor("d_xtil", [128, CT, N], BF16, kind="ExternalOutput")
        d_KT = nc.dram_tensor("d_KT", [128, CT, N], BF16, kind="ExternalOutput")
        d_QT = nc.dram_tensor("d_QT", [128, CT, NQ], BF16, kind="ExternalOutput")
        d_V4 = nc.dram_tensor("d_V4", [128, NT, HEADS, D + 1], BF16, kind="ExternalOutput")
        d_AO = nc.dram_tensor("d_AO", [128, CT, NQ], BF16, kind="ExternalOutput")

    with tile.TileContext(nc) as tc:
        with (
            tc.tile_pool(name="persist", bufs=1) as P1,
            tc.tile_pool(name="work", bufs=2) as PW,
            tc.tile_pool(name="rows", bufs=2) as PR,
            tc.tile_pool(name="ps", bufs=4, space="PSUM") as PS,
        ):
            # ---- persistent SBUF tensors ----
            wq = P1.tile([128, CT, 3 * DIM], BF16)       # W'' rows 0..767
            wex = P1.tile([2, 3 * DIM], BF16)            # W'' rows 768..769
            WO = P1.tile([128, CT, DIM], BF16)           # w_out^T  (f-major tiles)
            wob = P1.tile([1, DIM], BF16)                # b_out row
            xtil = P1.tile([128, CT, N], BF16, tag="big_a")   # x~ = x^T * rstd
            xex = P1.tile([2, N], BF16)                  # x~ rows 768 (-mu*rstd), 769 (1)
            KT = P1.tile([128, CT, N], BF16)             # K^T channel-major
            QT = P1.tile([128, CT, NQ], BF16)            # Q^T channel-major
            V4 = P1.tile([128, NT, HEADS, D + 1], BF16)  # V token-major + ones col
            ones1 = P1.tile([128, 1], F32)               # f32 ones column (stats lhsT)
            ones1b = P1.tile([128, 1], BF16)             # bf16 ones column
            onesr = P1.tile([1, 128], F32)               # f32 ones row (bcast lhsT)
            onesI = P1.tile([1, NQ], BF16)               # bf16 ones row (bias rhs)
            epsc = P1.tile([1, 1], F32)

            nc.vector.memset(epsc[:], 1e-5)
            nc.vector.memset(ones1[:], 1.0)
            nc.vector.memset(ones1b[:], 1.0)
            nc.vector.memset(onesr[:], 1.0)
            nc.vector.memset(onesI[:], 1.0)
            # row 1 must stay 1.0; row 0 is overwritten with -mu*rstd in phase B
            nc.vector.memset(xex[:, :], 1.0)
            nc.vector.memset(V4[:, :, :, D : D + 1], 1.0)

            # ---- weight DMAs ----
            for ct in range(CT):
                nc.sync.dma_start(wq[:, ct, :], wqkvT[ct * 128 : (ct + 1) * 128, :])
                nc.sync.dma_start(WO[:, ct, :], woutT[ct * 128 : (ct + 1) * 128, :])
            nc.sync.dma_start(wex[:], wqkvT[DIM : DIM + 2, :])
            nc.sync.dma_start(wob[:], woutT[DIM : DIM + 1, :])

            # ---- phase A: token stats (sum, sumsq) via ones-matmul reduction ----
            # rows 0=sum, 64=sumsq, 32=mu^2 scratch; one tile per 1024-column half
            sts = [PS.tile([128, 1024], F32, tag="ps", name=f"st_{h}") for h in range(2)]
            for ct in range(CT):
                xin = PW.tile([128, N], F32, tag="xin", name=f"xinA_{ct}")
                nc.sync.dma_start(xin[:], xT[ct * 128 : (ct + 1) * 128, :])
                xsq = PW.tile([128, N], BF16, tag="xsq")
                nc.scalar.activation(xsq[:], xin[:], AF.Square)
                for jc in range(4):
                    st = sts[jc // 2]
                    osl = slice((jc % 2) * 512, (jc % 2 + 1) * 512)
                    sl = slice(jc * 512, (jc + 1) * 512)
                    nc.tensor.matmul(
                        st[0:1, osl], ones1[:], xin[:, sl],
                        start=(ct == 0), stop=(ct == CT - 1),
                    )
                    nc.tensor.matmul(
                        st[64:65, osl], ones1b[:], xsq[:, sl],
                        start=(ct == 0), stop=(ct == CT - 1),
                    )

            r_a = PR.tile([1, N], F32, tag="row")   # E[x^2] -> var -> rstd
            r_b = PR.tile([1, N], F32, tag="row")   # mu -> mu*rstd
            for h in range(2):
                hsl = slice(h * 1024, (h + 1) * 1024)
                nc.vector.tensor_scalar_mul(r_a[:, hsl], sts[h][64:65, :], 1.0 / DIM)
                nc.vector.tensor_scalar_mul(r_b[:, hsl], sts[h][0:1, :], 1.0 / DIM)
                nc.vector.tensor_tensor(sts[h][32:33, :], r_b[:, hsl], r_b[:, hsl], ALU.mult)
                nc.vector.tensor_tensor(r_a[:, hsl], r_a[:, hsl], sts[h][32:33, :], ALU.subtract)
            # rstd = exp(-0.5 * ln(var + eps))  (ln+exp share one ACT table set)
            nc.scalar.activation(r_a[:], r_a[:], AF.Ln, bias=epsc[:])
            nc.scalar.activation(r_a[:], r_a[:], AF.Exp, scale=-0.5)
            if debug:
                nc.sync.dma_start(d_rows[0:1, :], r_a[:])
                nc.sync.dma_start(d_rows[1:2, :], r_b[:])

            # ---- phase B: x~ = x * rstd (bf16), plus folded-LN extra rows ----
            ps_rbs = [PS.tile([128, 1024], F32, tag="ps", name=f"rb_{h}") for h in range(2)]
            for jc in range(4):
                sl = slice(jc * 512, (jc + 1) * 512)
                nc.tensor.matmul(
                    ps_rbs[jc // 2][:, (jc % 2) * 512 : (jc % 2 + 1) * 512],
                    onesr[:], r_a[:, sl],
                )
            for ct in range(CT):
                xin = PW.tile([128, N], F32, tag="xin", name=f"xinB_{ct}")
                nc.sync.dma_start(xin[:], xT[ct * 128 : (ct + 1) * 128, :])
                for h in range(2):
                    hsl = slice(h * 1024, (h + 1) * 1024)
                    nc.vector.tensor_tensor(
                        xtil[:, ct, hsl], xin[:, hsl], ps_rbs[h][:], ALU.mult
                    )
            nc.vector.tensor_tensor(r_b[:], r_b[:], r_a[:], ALU.mult)
            nc.vector.tensor_scalar_mul(xex[0:1, :], r_b[:], -1.0)
            if debug:
                for ct in range(CT):
                    nc.sync.dma_start(d_xtil[:, ct, :], xtil[:, ct, :])

            # ---- phase C: QKV projections ----
            # K then Q (channel-major): out[f, n] ; lhsT = W'' tile, rhs = x~
            for ft in range(CT):
                fsl = slice(DIM + ft * 128, DIM + (ft + 1) * 128)
                for nh in range(2):
                    acc = PS.tile([128, 1024], F32, tag="ps")
                    for ct in range(CT + 1):
                        for s in range(2):
                            nsl = slice(nh * 1024 + s * 512, nh * 1024 + (s + 1) * 512)
                            nc.tensor.matmul(
                                acc[:, s * 512 : (s + 1) * 512],
                                wq[:, ct, fsl] if ct < CT else wex[:, fsl],
                                xtil[:, ct, nsl] if ct < CT else xex[:, nsl],
                                start=(ct == 0), stop=(ct == CT),
                            )
                    nc.vector.tensor_copy(KT[:, ft, nh * 1024 : (nh + 1) * 1024], acc[:])
            for ft in range(CT):
                fsl = slice(ft * 128, (ft + 1) * 128)
                acc = PS.tile([128, 1024], F32, tag="ps")
                for ct in range(CT + 1):
                    for s in range(2):
                        nsl = slice(s * 512, (s + 1) * 512)
                        nc.tensor.matmul(
                            acc[:, nsl],
                            wq[:, ct, fsl] if ct < CT else wex[:, fsl],
                            xtil[:, ct, nsl] if ct < CT else xex[:, nsl],
                            start=(ct == 0), stop=(ct == CT),
                        )
                nc.vector.tensor_copy(QT[:, ft, :], acc[:])
            # V (token-major): out[n, f] ; lhsT = x~ tile, rhs = W'' v-columns
            for nt in range(NT):
                nsl = slice(nt * 128, (nt + 1) * 128)
                acc = PS.tile([128, 1024], F32, tag="ps")
                for ct in range(CT + 1):
                    # matmul output must not straddle a 2KB PSUM bank: 768 = 512 + 256
                    for lo, sz in ((0, 512), (512, 256)):
                        fsl = slice(2 * DIM + lo, 2 * DIM + lo + sz)
                        nc.tensor.matmul(
                            acc[:, lo : lo + sz],
                            xtil[:, ct, nsl] if ct < CT else xex[:, nsl],
                            wq[:, ct, fsl] if ct < CT else wex[:, fsl],
                            start=(ct == 0), stop=(ct == CT),
                        )
                nc.vector.tensor_copy(
                    V4[:, nt, :, 0:D],
                    acc[:, 0:DIM].rearrange("p (h d) -> p h d", h=HEADS),
                )

            if debug:
                for ct in range(CT):
                    nc.sync.dma_start(d_KT[:, ct, :], KT[:, ct, :])
                    nc.sync.dma_start(d_QT[:, ct, :], QT[:, ct, :])
                for nt in range(NT):
                    nc.sync.dma_start(d_V4[:, nt, :, :], V4[:, nt, :, :])

            # attention output, same f-major tile layout the out-proj consumes
            AO = P1.tile([128, CT, NQ], BF16, tag="big_a", name="AO")

            # ---- phase D: attention, one head-pair at a time ----
            for hp in range(HP):
                U0 = PS.tile([128, 1024], F32, tag="ps", name=f"U0_{hp}")
                U1 = PS.tile([128, 1024], F32, tag="ps", name=f"U1_{hp}")
                for jt in range(NT):
                    jsl = slice(jt * 128, (jt + 1) * 128)
                    for h01 in range(2):
                        psl = slice(h01 * 64, (h01 + 1) * 64)
                        sp = PS.tile([128, 1024], F32, tag="ps", name=f"sp_{hp}_{jt}_{h01}")
                        for s in range(2):
                            ssl = slice(s * 512, (s + 1) * 512)
                            nc.tensor.matmul(
                                sp[:, ssl], KT[psl, hp, jsl], QT[psl, hp, ssl],
                                start=True, stop=True,
                            )
                        ET = PW.tile([128, 1024], BF16, tag="ET", name=f"ET_{hp}_{jt}_{h01}")
                        nc.scalar.activation(ET[:], sp[:], AF.Exp, scale=float(D) ** -0.5)
                        U = U0 if h01 == 0 else U1
                        for s in range(2):
                            ssl = slice(s * 512, (s + 1) * 512)
                            nc.tensor.matmul(
                                U[0 : D + 1, ssl], V4[:, jt, 2 * hp + h01, :], ET[:, ssl],
                                start=(jt == 0), stop=(jt == NT - 1),
                            )
                # r = 1/denominator via exp(-ln(s)) on ACT (vector.reciprocal is
                # ~8 cyc/elem iterative divide -- far slower), then partition-
                # broadcast by DMA (engines stay free)
                rr = PR.tile([1, N], F32, tag="row", name=f"rr_{hp}")
                nc.scalar.activation(rr[0:1, 0:1024], U0[D : D + 1, :], AF.Ln)
                nc.scalar.activation(rr[0:1, 1024:2048], U1[D : D + 1, :], AF.Ln)
                rrb = PW.tile([1, N], BF16, tag="rrb", name=f"rrb_{hp}")
                nc.scalar.activation(rrb[:], rr[:], AF.Exp, scale=-1.0)
                rbB = PW.tile([64, N], BF16, tag="rbB")
                nc.gpsimd.partition_broadcast(rbB[:], rrb[:])
                nc.vector.tensor_tensor(
                    AO[0:64, hp, :], U0[0:D, :], rbB[:, 0:1024], ALU.mult
                )
                AOtmp = PW.tile([64, NQ], BF16, tag="AOtmp", name=f"AOtmp_{hp}")
                nc.vector.tensor_tensor(
                    AOtmp[:], U1[0:D, :], rbB[:, 1024:2048], ALU.mult
                )
                nc.sync.dma_start(AO[64:128, hp, :], AOtmp[:])

            # ---- phase E: output projection + bias ----
            for ot in range(CT):
                osl = slice(ot * 128, (ot + 1) * 128)
                po = PS.tile([128, 1024], F32, tag="ps", name=f"po_{ot}")
                for s in range(2):
                    ssl = slice(s * 512, (s + 1) * 512)
                    nc.tensor.matmul(
                        po[:, ssl], wob[:, osl], onesI[:, ssl],
                        start=True, stop=False,
                    )
                    for ft in range(CT):
                        nc.tensor.matmul(
                            po[:, ssl], WO[:, ft, osl], AO[:, ft, ssl],
                            start=False, stop=(ft == CT - 1),
                        )
                outsb = PW.tile([128, 1024], F32, tag="outsb")
                nc.vector.tensor_copy(outsb[:], po[:])
                nc.sync.dma_start(outT[osl, :], outsb[:])

    nc.finalize()
    return nc


def _get_nc():
    global _NC
    if _NC is None:
        import os
        _NC = build(debug=os.environ.get("KDEBUG", "0") == "1")
    return _NC


def kernel(x, ln_w, ln_b, w_qkv, w_out, b_out):
    global LAST
    x = np.asarray(x, dtype=np.float32)
    ln_w = np.asarray(ln_w, dtype=np.float32)
    ln_b = np.asarray(ln_b, dtype=np.float32)
    w_qkv = np.asarray(w_qkv, dtype=np.float32)
    w_out = np.asarray(w_out, dtype=np.float32)
    b_out = np.asarray(b_out, dtype=np.float32)

    bf16 = ml_dtypes.bfloat16
    # W'' = [ (w_qkv * ln_w)^T ; rowsum of (w_qkv*ln_w) ; w_qkv @ ln_b ]
    wprime = w_qkv * ln_w[None, :]
    wqkvT = np.concatenate(
        [wprime.T, wprime.sum(axis=1)[None, :], (w_qkv @ ln_b)[None, :]], axis=0
    ).astype(bf16)
    woutT = np.concatenate([w_out.T, b_out[None, :]], axis=0).astype(bf16)

    in_maps = []
    for c in range(8):
        b, g = c // 2, c % 2
        order = np.r_[g * NQ : (g + 1) * NQ, (1 - g) * NQ : (2 - g) * NQ]
        xTc = np.ascontiguousarray(x[b][order].T)
        in_maps.append({"xT": xTc, "wqkvT": wqkvT, "woutT": woutT})

    nc = _get_nc()
    LAST = run_bass_kernel_spmd(nc, in_maps, core_ids=list(range(8)))

    out = np.empty((B, N, DIM), dtype=np.float32)
    for c in range(8):
        b, g = c // 2, c % 2
        out[b, g * NQ : (g + 1) * NQ, :] = LAST.results[c]["outT"].T
    return out
